# revision 16
# baseline (speedup 1.0000x reference)
"""Multi-head attention (dense transformer block) on 8 Trainium2 NeuronCores.

Reference computation (per batch element b of 8):
    qkv = x @ w_qkv.T + b_qkv                  # [1024, 2304]
    q, k, v = split heads (12 heads, d=64)
    attn = softmax(q k^T / sqrt(d))
    out  = (attn v) reshaped @ w_proj.T + b_proj
Sharding: pure data parallel — core b handles batch element b, weights are
replicated, no collectives.

Per-core kernel (all matmul operands fp16, fp32 PSUM accumulation):
  C: v    = x Wv^T + b_v            -> [1024, 12*(64+1)] (ones col per head
     makes the PV matmul emit softmax row-sums for free)
  B: qk^T = [Wq*scale; Wk] x^T      -> [1536, 1024] (features on partitions)
  D: per head pair hp: scores^T = k^T q (two heads row-tiled into the two
     halves of the PE array), exp on ScalarE straight from PSUM pairs,
     PV accumulate, normalize by approx-reciprocal(rowsum).
  E: out = score w_proj^T + b_proj  (bias via pre-broadcast rows on DVE)

Schedule notes (v2):
  - All DRAM inputs are host-prearranged partition-major so every DMA is a
    flat 2-3 level descriptor; issue is split across both HWDGE issuing
    engines (sync: x + wv, scalar: wqk + wp + biases) with the pieces the
    first matmuls need queued first, so the PE starts ~5us earlier.
  - B(0)/B(6) run first (their data lands first); all C groups become
    wave-0/1 fillers.
  - pv fillers are spread through each wave (alternating with B fillers)
    so their PSUM-evict chains never bunch up and stall the PE.
  - Wave 5 computes the nq=512 scores first so the pv(5) normalize chains
    are hidden under the first e-tiles; out staging+DMA is fp16 (host
    casts back to fp32).
"""

import os
import sys

for _p in ("/opt/trn_rl_repo", "/root/.axon_site/_ro/trn_rl_repo"):
    if os.path.isdir(_p) and _p not in sys.path:
        sys.path.insert(0, _p)

import numpy as np

import concourse.bass as bass
import concourse.mybir as mybir
import concourse.tile as tile
from concourse import bacc
from concourse.bass_utils import run_bass_kernel_spmd

DIM = 768
N_HEAD = 12
HEAD_DIM = 64
SCALE = HEAD_DIM ** (-0.5)
NB = 8          # batch == number of cores
N = 1024        # sequence length
CCH = DIM // 128  # 6 contraction chunks

F32 = mybir.dt.float32
F16 = mybir.dt.float16
AF = mybir.ActivationFunctionType

_CACHE: dict = {}


def _build():
    nc = bacc.Bacc("TRN2", target_bir_lowering=False, debug=False)

    # partition-major inputs, pre-arranged on host so every DMA below reads
    # and writes fully contiguous per-partition runs:
    #   x_h   [p, nq, c, 512]   wqk_h [p, ot, c, 128]
    #   wv_h / wp_h: blk0 [p, c, 512] then blk1 [p, c, 256], flattened
    x_d = nc.dram_tensor("x_p", [128, CCH * N], F16, kind="ExternalInput")
    wqk_d = nc.dram_tensor("wqk_p", [128, CCH * 2 * DIM], F16, kind="ExternalInput")
    wv_d = nc.dram_tensor("wv_p", [128, CCH * DIM], F16, kind="ExternalInput")
    wp_d = nc.dram_tensor("wp_p", [128, CCH * DIM], F16, kind="ExternalInput")
    bqk_d = nc.dram_tensor("b_qk_t", [128, 12], F32, kind="ExternalInput")
    bvp_d = nc.dram_tensor("b_vp", [1, 2 * DIM], F16, kind="ExternalInput")
    out_d = nc.dram_tensor("out", [N, DIM], F16, kind="ExternalOutput")

    with tile.TileContext(nc) as tc:
        with (
            tc.tile_pool(name="consts", bufs=1) as consts,
            tc.tile_pool(name="qk", bufs=1) as qk_pool,
            tc.tile_pool(name="score", bufs=1) as score_pool,
            tc.tile_pool(name="v", bufs=1) as v_pool,
            tc.tile_pool(name="x", bufs=1) as x_pool,
            tc.tile_pool(name="wqk", bufs=1) as wqk_pool,
            tc.tile_pool(name="wv", bufs=1) as wv_pool,
            tc.tile_pool(name="wp", bufs=1) as wp_pool,
            tc.tile_pool(name="attn", bufs=32) as attn_pool,
            tc.tile_pool(name="small", bufs=4) as small_pool,
            tc.tile_pool(name="ostage", bufs=2) as out_pool,
            tc.tile_pool(name="ps", bufs=2, space="PSUM") as ps_pool,
            tc.tile_pool(name="pair", bufs=2, space="PSUM") as pair_pool,
            tc.tile_pool(name="acc", bufs=2, space="PSUM") as acc_pool,
        ):
            x_sb = x_pool.tile([128, 2, CCH, 512], F16)      # [p, nq, c, n]
            wqk_sb = wqk_pool.tile([128, 12, CCH, 128], F16)  # [p, ot, c, o]
            wva_sb = wv_pool.tile([128, CCH, 512], F16)
            wvb_sb = wv_pool.tile([128, CCH, 256], F16)
            wpa_sb = wp_pool.tile([128, CCH, 512], F16)
            wpb_sb = wp_pool.tile([128, CCH, 256], F16)
            bqk_sb = consts.tile([128, 12], F32)
            bvp_sb = consts.tile([1, 2 * DIM], F16)

            # scalar-engine HWDGE queue: the qk-projection weights the first
            # B groups need, then biases + v weights, then the rest
            wqk_v = wqk_d[:].rearrange("p (t c o) -> p t c o", t=12, c=CCH)
            wv_v = wv_d[:]
            wp_v = wp_d[:]
            nc.scalar.dma_start(wqk_sb[:, 0], wqk_v[:, 0])
            nc.scalar.dma_start(wqk_sb[:, 6], wqk_v[:, 6])
            nc.scalar.dma_start(bqk_sb[:], bqk_d[:])
            nc.scalar.dma_start(bvp_sb[:], bvp_d[:])
            nc.scalar.dma_start(
                wva_sb[:], wv_v[:, 0:CCH * 512].rearrange("p (c o) -> p c o", c=CCH))
            nc.scalar.dma_start(wqk_sb[:, 1:6], wqk_v[:, 1:6])
            nc.scalar.dma_start(wqk_sb[:, 7:12], wqk_v[:, 7:12])
            nc.scalar.dma_start(
                wpa_sb[:], wp_v[:, 0:CCH * 512].rearrange("p (c o) -> p c o", c=CCH))
            nc.scalar.dma_start(
                wpb_sb[:],
                wp_v[:, CCH * 512:CCH * DIM].rearrange("p (c o) -> p c o", c=CCH))

            # sync-engine HWDGE queue: x (first B group's c-chunks first)
            x_v = x_d[:].rearrange("p (q c n) -> p q c n", q=2, c=CCH)
            nc.sync.dma_start(x_sb[:, 0, 0:3], x_v[:, 0, 0:3])
            nc.sync.dma_start(x_sb[:, 0, 3:CCH], x_v[:, 0, 3:CCH])
            nc.sync.dma_start(x_sb[:, 1], x_v[:, 1])
            nc.sync.dma_start(
                wvb_sb[:],
                wv_v[:, CCH * 512:CCH * DIM].rearrange("p (c o) -> p c o", c=CCH))

            # broadcast b_v / b_p across partitions once; the evictions add
            # them on the DVE, saving 32 ones-row bias matmuls on the PE
            bv32 = consts.tile([1, DIM], F32)
            nc.vector.tensor_copy(bv32[:], bvp_sb[:, 0:DIM])
            bvb = consts.tile([128, DIM], F32)
            nc.gpsimd.partition_broadcast(bvb[:], bv32[:], channels=128)
            bp32 = consts.tile([1, DIM], F32)
            nc.vector.tensor_copy(bp32[:], bvp_sb[:, DIM:2 * DIM])
            bpb = consts.tile([128, DIM], F32)
            nc.gpsimd.partition_broadcast(bpb[:], bp32[:], channels=128)

            qk_sb = qk_pool.tile([128, 12, N], F16)         # [o=1536, n]
            score_sb = score_pool.tile([128, CCH, N], F16)  # [c=768, n]
            v_sb = v_pool.tile([128, 8, N_HEAD * 65], F16)  # [n, h*(64+1)]

            # ---- Phase C: v projection, natural layout + ones cols ----
            v_ones = v_sb[:].rearrange("p n (h d) -> p n h d", d=65)[:, :, :, 64:65]
            nc.vector.memset(v_ones, 1.0)

            def c_group(nt, blk):
                o0, ow, off = ((0, 512, 0), (512, 256, 8 * 65))[blk]
                wblk = (wva_sb, wvb_sb)[blk]
                nqi, n0 = nt // 4, (nt % 4) * 128
                ps = ps_pool.tile([128, 512], F32)
                for c in range(CCH):
                    nc.tensor.matmul(
                        ps[:, :ow],
                        x_sb[:, nqi, c, n0:n0 + 128],
                        wblk[:, c, :],
                        start=(c == 0),
                        stop=(c == CCH - 1),
                    )
                nh = ow // 64
                src = ps[:, :ow].rearrange("p (h d) -> p h d", d=64)
                bias = bvb[:, o0:o0 + ow].rearrange("p (h d) -> p h d", d=64)
                dst = v_sb[:, nt, off:off + nh * 65].rearrange(
                    "p (h d) -> p h d", d=65
                )[:, :, 0:64]
                nc.vector.tensor_add(dst, src, bias)

            # ---- Phase B helper: one [o-tile, nq] strip of the qk^T proj ----
            def b_group(ot, nq):
                ps = ps_pool.tile([128, 512], F32)
                for c in range(CCH):
                    nc.tensor.matmul(
                        ps[:],
                        wqk_sb[:, ot, c, :],
                        x_sb[:, nq // 512, c, :],
                        start=(c == 0),
                        stop=(c == CCH - 1),
                    )
                nc.vector.tensor_scalar_add(
                    qk_sb[:, ot, nq:nq + 512], ps[:], bqk_sb[:, ot:ot + 1],
                )

            # ---- Phase D helpers ----
            def score_pair(hp, nq, nk):
                """scoresT for both heads of pair hp, one nk tile: head A into
                cols 0:512 (PE rows 0-63), head B into 512:1024 (rows 64-127),
                then exp straight from the 2-bank PSUM pair into fp16 SBUF."""
                pair = pair_pool.tile([128, 1024], F32)
                for half, p0 in ((0, 0), (1, 64)):
                    nc.tensor.matmul(
                        pair[:, half * 512:(half + 1) * 512],
                        qk_sb[p0:p0 + 64, 6 + hp, nk * 128:(nk + 1) * 128],
                        qk_sb[p0:p0 + 64, hp, nq:nq + 512],
                        start=True, stop=True,
                        tile_position=(p0, 0),
                    )
                at = attn_pool.tile([128, 1024], F16)
                nc.scalar.activation(at[:], pair[:], AF.Exp)
                return at

            def pv_group(hp, nq, half, p0, attns):
                """attn @ [v|1] for one head/nq strip + normalize by rowsum."""
                h = 2 * hp + half
                acc = acc_pool.tile([65, 512], F32)
                for nk in range(8):
                    nc.tensor.matmul(
                        acc[:],
                        v_sb[:, nk, h * 65:(h + 1) * 65],
                        attns[nk][:, half * 512:(half + 1) * 512],
                        start=(nk == 0),
                        stop=(nk == 7),
                    )
                # custom-DVE ops mis-read PSUM APs at partition offsets > 0 —
                # stage the rowsum row to SBUF first.
                rs = small_pool.tile([1, 512], F32, tag="rs")
                nc.vector.tensor_copy(rs[:], acc[64:65, :])
                rec = small_pool.tile([1, 512], F32, tag="rec")
                nc.vector.reciprocal_approx_fast(rec[:], rs[:])
                bc = small_pool.tile([64, 512], F32, tag="bc")
                nc.gpsimd.partition_broadcast(bc[:], rec[:], channels=64)
                nc.vector.tensor_mul(
                    score_sb[p0:p0 + 64, hp, nq:nq + 512], acc[0:64, :], bc[:],
                )

            # ---- Phases B + D interleaved in waves over head pairs ----
            b_group(0, 0)
            b_group(6, 0)
            b_group(0, 512)
            b_group(6, 512)
            prev_strips = None
            for hp in range(5):
                # fillers keep the PE busy while ScalarE exps this wave;
                # pv groups are spread between B/C groups so their evict
                # chains overlap non-pv PE work
                cg, bg, pg = [], [], []
                if hp == 0:
                    for nt in range(6):
                        for blk in (0, 1):
                            cg.append(lambda nt=nt, blk=blk: c_group(nt, blk))
                elif hp == 1:
                    # remaining v-proj tiles run before this wave's pv(0)
                    # fillers, so v_sb is complete when PV needs it
                    for nt in (6, 7):
                        for blk in (0, 1):
                            cg.append(lambda nt=nt, blk=blk: c_group(nt, blk))
                for nq in (0, 512):
                    for ot in (hp + 1, 7 + hp):
                        bg.append(lambda ot=ot, nq=nq: b_group(ot, nq))
                if prev_strips is not None:
                    php, pstrips = prev_strips
                    for nq in (0, 512):
                        for half, p0 in ((0, 0), (1, 64)):
                            pg.append(
                                lambda nq=nq, half=half, p0=p0, php=php,
                                       s=pstrips: pv_group(php, nq, half, p0, s[nq])
                            )
                fillers = list(cg)
                # alternate pv / b so pv evict chains are spaced out
                while pg or bg:
                    if pg:
                        fillers.append(pg.pop(0))
                    if bg:
                        fillers.append(bg.pop(0))
                strips = {0: [], 512: []}
                fi = 0
                for nq in (0, 512):
                    for nk in range(8):
                        strips[nq].append(score_pair(hp, nq, nk))
                        if fi < len(fillers):
                            fillers[fi]()
                            fi += 1
                while fi < len(fillers):
                    fillers[fi]()
                    fi += 1
                prev_strips = (hp, strips)

            # ---- Phase E helper ----
            def e_tile(nt):
                stage = out_pool.tile([128, DIM], F16)
                for blk, (o0, ow) in enumerate(((0, 512), (512, 256))):
                    wblk = (wpa_sb, wpb_sb)[blk]
                    ps = ps_pool.tile([128, 512], F32)
                    for c in range(CCH):
                        nc.tensor.matmul(
                            ps[:, :ow],
                            score_sb[:, c, nt * 128:(nt + 1) * 128],
                            wblk[:, c, :],
                            start=(c == 0),
                            stop=(c == CCH - 1),
                        )
                    nc.vector.tensor_add(
                        stage[:, o0:o0 + ow], ps[:, :ow], bpb[:, o0:o0 + ow],
                    )
                    nc.sync.dma_start(
                        out_d[nt * 128:(nt + 1) * 128, o0:o0 + ow],
                        stage[:, o0:o0 + ow],
                    )

            # ---- wave 5: nq=512 scores first (pv(4) fillers), then nq=0
            # scores (pv(5,512) fillers), so every pv(5) normalize chain is
            # hidden under score pairs or e-tiles ----
            _, p4 = prev_strips
            strips5 = {0: [], 512: []}
            for nk in range(8):
                strips5[512].append(score_pair(5, 512, nk))
                if nk == 1:
                    pv_group(4, 0, 0, 0, p4[0])
                elif nk == 3:
                    pv_group(4, 0, 1, 64, p4[0])
                elif nk == 5:
                    pv_group(4, 512, 0, 0, p4[512])
            pv_group(5, 512, 0, 0, strips5[512])
            for nk in range(8):
                strips5[0].append(score_pair(5, 0, nk))
                if nk == 1:
                    pv_group(5, 512, 1, 64, strips5[512])
                elif nk == 3:
                    pv_group(4, 512, 1, 64, p4[512])
            e_tile(4)
            pv_group(5, 0, 0, 0, strips5[0])
            e_tile(5)
            pv_group(5, 0, 1, 64, strips5[0])
            e_tile(6)
            e_tile(7)
            for nt in (0, 1, 2, 3):
                e_tile(nt)

    nc.compile()
    return nc


def _get_nc():
    if "nc" not in _CACHE:
        _CACHE["nc"] = _build()
    return _CACHE["nc"]


def _x_h(xT):
    """[768, 1024] -> [p, nq, c, 512] flattened fp16."""
    return np.ascontiguousarray(
        xT.reshape(CCH, 128, 2, 512).transpose(1, 2, 0, 3)
    ).reshape(128, CCH * N).astype(np.float16)


def _wqk_h(w):
    """[768, 1536] -> [p, ot, c, 128] flattened fp16."""
    return np.ascontiguousarray(
        w.reshape(CCH, 128, 12, 128).transpose(1, 2, 0, 3)
    ).reshape(128, CCH * 2 * DIM).astype(np.float16)


def _wblk_h(w):
    """[768, 768] -> blk0 [p, c, 512] ++ blk1 [p, c, 256] fp16."""
    b0 = w[:, 0:512].reshape(CCH, 128, 512).transpose(1, 0, 2).reshape(128, -1)
    b1 = w[:, 512:DIM].reshape(CCH, 128, 256).transpose(1, 0, 2).reshape(128, -1)
    return np.ascontiguousarray(
        np.concatenate([b0, b1], axis=1)).astype(np.float16)


def kernel(x, w_qkv, b_qkv, w_proj, b_proj, **run_kwargs):
    x = np.asarray(x, dtype=np.float32)
    w_qkv = np.asarray(w_qkv, dtype=np.float32)
    b_qkv = np.asarray(b_qkv, dtype=np.float32)
    w_proj = np.asarray(w_proj, dtype=np.float32)
    b_proj = np.asarray(b_proj, dtype=np.float32)

    # Host-side layout prep (no arithmetic beyond folding the 1/sqrt(d) scale
    # into the q projection).
    w_qk = w_qkv[: 2 * DIM].copy()
    b_qk = b_qkv[: 2 * DIM].copy()
    w_qk[:DIM] *= SCALE
    b_qk[:DIM] *= SCALE
    wqk_p = _wqk_h(w_qk.T)                             # [128, 6*1536]
    b_qk_t = np.ascontiguousarray(b_qk.reshape(12, 128).T)  # [128, 12] f32
    wv_p = _wblk_h(w_qkv[2 * DIM:].T)
    wp_p = _wblk_h(w_proj.T)
    b_vp = np.concatenate(
        [b_qkv[2 * DIM:], b_proj]).reshape(1, 2 * DIM).astype(np.float16)

    nc = _get_nc()
    in_maps = []
    for b in range(NB):
        in_maps.append({
            "x_p": _x_h(x[b].T),
            "wqk_p": wqk_p,
            "b_qk_t": b_qk_t,
            "wv_p": wv_p,
            "b_vp": b_vp,
            "wp_p": wp_p,
        })
    res = run_bass_kernel_spmd(nc, in_maps, core_ids=list(range(NB)), **run_kwargs)
    out = np.stack(
        [res.results[b]["out"] for b in range(NB)], axis=0).astype(np.float32)
    if run_kwargs:
        return out, res
    return out


if __name__ == "__main__":
    rng = np.random.default_rng(0)
    x = rng.standard_normal((NB, N, DIM), dtype=np.float32)
    w_qkv = rng.standard_normal((3 * DIM, DIM), dtype=np.float32) * DIM ** -0.5
    b_qkv = rng.standard_normal((3 * DIM,), dtype=np.float32) * 0.02
    w_proj = rng.standard_normal((DIM, DIM), dtype=np.float32) * DIM ** -0.5
    b_proj = rng.standard_normal((DIM,), dtype=np.float32) * 0.02
    out = kernel(x=x, w_qkv=w_qkv, b_qkv=b_qkv, w_proj=w_proj, b_proj=b_proj)
    print("out", out.shape, out.dtype, float(np.abs(out).mean()))


# revision 27
# speedup vs baseline: 1.0167x; 1.0167x over previous
"""Multi-head attention (dense transformer block) on 8 Trainium2 NeuronCores.

Reference computation (per batch element b of 8):
    qkv = x @ w_qkv.T + b_qkv                  # [1024, 2304]
    q, k, v = split heads (12 heads, d=64)
    attn = softmax(q k^T / sqrt(d))
    out  = (attn v) reshaped @ w_proj.T + b_proj
Sharding: pure data parallel — core b handles batch element b, weights are
replicated, no collectives.

Per-core kernel (all matmul operands fp16, fp32 PSUM accumulation):
  C: v    = x Wv^T + b_v            -> [1024, 12*(64+1)] (ones col per head
     makes the PV matmul emit softmax row-sums for free)
  B: qk^T = [Wq*scale; Wk] x^T      -> [1536, 1024] (features on partitions)
  D: per head pair hp: scores^T = k^T q (two heads row-tiled into the two
     halves of the PE array), exp on ScalarE straight from PSUM pairs,
     PV accumulate, normalize by approx-reciprocal(rowsum).
  E: out = score w_proj^T + b_proj  (bias via pre-broadcast rows on DVE)

Schedule notes (v2):
  - All DRAM inputs are host-prearranged partition-major so every DMA is a
    flat 2-3 level descriptor; issue is split across both HWDGE issuing
    engines (sync: x + wv, scalar: wqk + wp + biases) with the pieces the
    first matmuls need queued first, so the PE starts ~5us earlier.
  - B(0)/B(6) run first (their data lands first); all C groups become
    wave-0/1 fillers.
  - pv fillers are spread through each wave (alternating with B fillers)
    so their PSUM-evict chains never bunch up and stall the PE.
  - Wave 5 computes the nq=512 scores first so the pv(5) normalize chains
    are hidden under the first e-tiles; out staging+DMA is fp16 (host
    casts back to fp32).
"""

import os
import sys

for _p in ("/opt/trn_rl_repo", "/root/.axon_site/_ro/trn_rl_repo"):
    if os.path.isdir(_p) and _p not in sys.path:
        sys.path.insert(0, _p)

import numpy as np

import concourse.bass as bass
import concourse.mybir as mybir
import concourse.tile as tile
from concourse import bacc
from concourse.bass_utils import run_bass_kernel_spmd

DIM = 768
N_HEAD = 12
HEAD_DIM = 64
SCALE = HEAD_DIM ** (-0.5)
NB = 8          # batch == number of cores
N = 1024        # sequence length
CCH = DIM // 128  # 6 contraction chunks

F32 = mybir.dt.float32
F16 = mybir.dt.float16
AF = mybir.ActivationFunctionType

_CACHE: dict = {}


def _build():
    nc = bacc.Bacc("TRN2", target_bir_lowering=False, debug=False)

    # partition-major inputs, pre-arranged on host so every DMA below reads
    # and writes fully contiguous per-partition runs:
    #   x_h   [p, nq, c, 512]   wqk_h [p, ot, c, 128]
    #   wv_h / wp_h: blk0 [p, c, 512] then blk1 [p, c, 256], flattened
    x_d = nc.dram_tensor("x_p", [128, CCH * N], F16, kind="ExternalInput")
    wqk_d = nc.dram_tensor("wqk_p", [128, CCH * 2 * DIM], F16, kind="ExternalInput")
    wv_d = nc.dram_tensor("wv_p", [128, CCH * DIM], F16, kind="ExternalInput")
    wp_d = nc.dram_tensor("wp_p", [128, CCH * DIM], F16, kind="ExternalInput")
    # cols 0:12 = b_qk per o-tile, 12:18 = b_proj per o-tile
    bias_d = nc.dram_tensor("biases", [128, 18], F32, kind="ExternalInput")
    bv_d = nc.dram_tensor("b_v", [1, DIM], F16, kind="ExternalInput")
    out_d = nc.dram_tensor("outT", [DIM, N], F16, kind="ExternalOutput")

    with tile.TileContext(nc) as tc:
        with (
            tc.tile_pool(name="consts", bufs=1) as consts,
            tc.tile_pool(name="qk", bufs=1) as qk_pool,
            tc.tile_pool(name="score", bufs=1) as score_pool,
            tc.tile_pool(name="v", bufs=1) as v_pool,
            tc.tile_pool(name="x", bufs=1) as x_pool,
            tc.tile_pool(name="wqk", bufs=1) as wqk_pool,
            tc.tile_pool(name="wv", bufs=1) as wv_pool,
            tc.tile_pool(name="wp", bufs=1) as wp_pool,
            tc.tile_pool(name="attn", bufs=32) as attn_pool,
            tc.tile_pool(name="small", bufs=4) as small_pool,
            tc.tile_pool(name="ostage", bufs=1) as out_pool,
            tc.tile_pool(name="ps", bufs=2, space="PSUM") as ps_pool,
            tc.tile_pool(name="pair", bufs=2, space="PSUM") as pair_pool,
            tc.tile_pool(name="acc", bufs=2, space="PSUM") as acc_pool,
        ):
            x_sb = x_pool.tile([128, 2, CCH, 512], F16)      # [p, nq, c, n]
            wqk_sb = wqk_pool.tile([128, 12, CCH, 128], F16)  # [p, ot, c, o]
            wva_sb = wv_pool.tile([128, CCH, 512], F16)
            wvb_sb = wv_pool.tile([128, CCH, 256], F16)
            wp_sb = wp_pool.tile([128, CCH, CCH, 128], F16)   # [p, ot, c, o]
            bias_sb = consts.tile([128, 18], F32)
            bv_sb = consts.tile([1, DIM], F16)

            # scalar-engine HWDGE queue: the qk-projection weights the first
            # B groups need, then biases + v weights, then the rest
            wqk_v = wqk_d[:].rearrange("p (t c o) -> p t c o", t=12, c=CCH)
            wv_v = wv_d[:]
            nc.scalar.dma_start(wqk_sb[:, 0], wqk_v[:, 0])
            nc.scalar.dma_start(wqk_sb[:, 6], wqk_v[:, 6])
            nc.scalar.dma_start(bias_sb[:], bias_d[:])
            nc.scalar.dma_start(bv_sb[:], bv_d[:])
            nc.scalar.dma_start(
                wva_sb[:], wv_v[:, 0:CCH * 512].rearrange("p (c o) -> p c o", c=CCH))
            nc.scalar.dma_start(wqk_sb[:, 1:6], wqk_v[:, 1:6])
            nc.scalar.dma_start(wqk_sb[:, 7:12], wqk_v[:, 7:12])
            nc.scalar.dma_start(
                wp_sb[:], wp_d[:].rearrange("p (t c o) -> p t c o", t=CCH, c=CCH))

            # sync-engine HWDGE queue: x (first B group's c-chunks first)
            x_v = x_d[:].rearrange("p (q c n) -> p q c n", q=2, c=CCH)
            nc.sync.dma_start(x_sb[:, 0, 0:3], x_v[:, 0, 0:3])
            nc.sync.dma_start(x_sb[:, 0, 3:CCH], x_v[:, 0, 3:CCH])
            nc.sync.dma_start(x_sb[:, 1], x_v[:, 1])
            nc.sync.dma_start(
                wvb_sb[:],
                wv_v[:, CCH * 512:CCH * DIM].rearrange("p (c o) -> p c o", c=CCH))

            # broadcast b_v across partitions once; the C evictions add it on
            # the DVE, saving ones-row bias matmuls on the PE
            bv32 = consts.tile([1, DIM], F32)
            nc.vector.tensor_copy(bv32[:], bv_sb[:])
            bvb = consts.tile([128, DIM], F32)
            nc.gpsimd.partition_broadcast(bvb[:], bv32[:], channels=128)

            qk_sb = qk_pool.tile([128, 12, N], F16)         # [o=1536, n]
            score_sb = score_pool.tile([128, CCH, N], F16)  # [c=768, n]
            v_sb = v_pool.tile([128, 8, N_HEAD * 65], F16)  # [n, h*(64+1)]

            # ---- Phase C: v projection, natural layout + ones cols ----
            v_ones = v_sb[:].rearrange("p n (h d) -> p n h d", d=65)[:, :, :, 64:65]
            nc.vector.memset(v_ones, 1.0)

            def c_group(nt, blk):
                o0, ow, off = ((0, 512, 0), (512, 256, 8 * 65))[blk]
                wblk = (wva_sb, wvb_sb)[blk]
                nqi, n0 = nt // 4, (nt % 4) * 128
                ps = ps_pool.tile([128, 512], F32)
                for c in range(CCH):
                    nc.tensor.matmul(
                        ps[:, :ow],
                        x_sb[:, nqi, c, n0:n0 + 128],
                        wblk[:, c, :],
                        start=(c == 0),
                        stop=(c == CCH - 1),
                    )
                nh = ow // 64
                src = ps[:, :ow].rearrange("p (h d) -> p h d", d=64)
                bias = bvb[:, o0:o0 + ow].rearrange("p (h d) -> p h d", d=64)
                dst = v_sb[:, nt, off:off + nh * 65].rearrange(
                    "p (h d) -> p h d", d=65
                )[:, :, 0:64]
                nc.vector.tensor_add(dst, src, bias)

            # ---- Phase B helper: one [o-tile, nq] strip of the qk^T proj ----
            def b_group(ot, nq):
                ps = ps_pool.tile([128, 512], F32)
                for c in range(CCH):
                    nc.tensor.matmul(
                        ps[:],
                        wqk_sb[:, ot, c, :],
                        x_sb[:, nq // 512, c, :],
                        start=(c == 0),
                        stop=(c == CCH - 1),
                    )
                nc.vector.tensor_scalar_add(
                    qk_sb[:, ot, nq:nq + 512], ps[:], bias_sb[:, ot:ot + 1],
                )

            # ---- Phase D helpers ----
            def score_pair(hp, nq, nk):
                """scoresT for both heads of pair hp, one nk tile: head A into
                cols 0:512 (PE rows 0-63), head B into 512:1024 (rows 64-127),
                then exp straight from the 2-bank PSUM pair into fp16 SBUF."""
                pair = pair_pool.tile([128, 1024], F32)
                for half, p0 in ((0, 0), (1, 64)):
                    nc.tensor.matmul(
                        pair[:, half * 512:(half + 1) * 512],
                        qk_sb[p0:p0 + 64, 6 + hp, nk * 128:(nk + 1) * 128],
                        qk_sb[p0:p0 + 64, hp, nq:nq + 512],
                        start=True, stop=True,
                        tile_position=(p0, 0),
                    )
                at = attn_pool.tile([128, 1024], F16)
                nc.scalar.activation(at[:], pair[:], AF.Exp)
                return at

            def pv_group(hp, nq, half, p0, attns):
                """attn @ [v|1] for one head/nq strip + normalize by rowsum."""
                h = 2 * hp + half
                acc = acc_pool.tile([65, 512], F32)
                for nk in range(8):
                    nc.tensor.matmul(
                        acc[:],
                        v_sb[:, nk, h * 65:(h + 1) * 65],
                        attns[nk][:, half * 512:(half + 1) * 512],
                        start=(nk == 0),
                        stop=(nk == 7),
                    )
                # custom-DVE ops mis-read PSUM APs at partition offsets > 0 —
                # stage the rowsum row to SBUF first.
                rs = small_pool.tile([1, 512], F32, tag="rs")
                nc.vector.tensor_copy(rs[:], acc[64:65, :])
                rec = small_pool.tile([1, 512], F32, tag="rec")
                nc.vector.reciprocal_approx_fast(rec[:], rs[:])
                bc = small_pool.tile([64, 512], F32, tag="bc")
                nc.gpsimd.partition_broadcast(bc[:], rec[:], channels=64)
                nc.vector.tensor_mul(
                    score_sb[p0:p0 + 64, hp, nq:nq + 512], acc[0:64, :], bc[:],
                )

            # ---- Phases B + D interleaved in waves over head pairs ----
            b_group(0, 0)
            b_group(6, 0)
            b_group(0, 512)
            b_group(6, 512)
            prev_strips = None
            for hp in range(5):
                # fillers keep the PE busy while ScalarE exps this wave;
                # pv groups are spread between B/C groups so their evict
                # chains overlap non-pv PE work
                cg, bg, pg = [], [], []
                if hp == 0:
                    for nt in range(6):
                        for blk in (0, 1):
                            cg.append(lambda nt=nt, blk=blk: c_group(nt, blk))
                elif hp == 1:
                    # remaining v-proj tiles run before this wave's pv(0)
                    # fillers, so v_sb is complete when PV needs it
                    for nt in (6, 7):
                        for blk in (0, 1):
                            cg.append(lambda nt=nt, blk=blk: c_group(nt, blk))
                for nq in (0, 512):
                    for ot in (hp + 1, 7 + hp):
                        bg.append(lambda ot=ot, nq=nq: b_group(ot, nq))
                if prev_strips is not None:
                    php, pstrips = prev_strips
                    for nq in (0, 512):
                        for half, p0 in ((0, 0), (1, 64)):
                            pg.append(
                                lambda nq=nq, half=half, p0=p0, php=php,
                                       s=pstrips: pv_group(php, nq, half, p0, s[nq])
                            )
                fillers = list(cg)
                # alternate pv / b so pv evict chains are spaced out
                while pg or bg:
                    if pg:
                        fillers.append(pg.pop(0))
                    if bg:
                        fillers.append(bg.pop(0))
                strips = {0: [], 512: []}
                fi = 0
                for nq in (0, 512):
                    for nk in range(8):
                        strips[nq].append(score_pair(hp, nq, nk))
                        if fi < len(fillers):
                            fillers[fi]()
                            fi += 1
                while fi < len(fillers):
                    fillers[fi]()
                    fi += 1
                prev_strips = (hp, strips)

            # ---- Phase E: transposed out-proj, one [o-tile, nq] unit at a
            # time: wp tile stationary, score moving, bias per-partition ----
            outT_sb = out_pool.tile([128, CCH, N], F16)

            def e_unit(ot, nq):
                ps = ps_pool.tile([128, 512], F32)
                for c in range(CCH):
                    nc.tensor.matmul(
                        ps[:],
                        wp_sb[:, ot, c, :],
                        score_sb[:, c, nq:nq + 512],
                        start=(c == 0),
                        stop=(c == CCH - 1),
                    )
                nc.vector.tensor_scalar_add(
                    outT_sb[:, ot, nq:nq + 512], ps[:], bias_sb[:, 12 + ot:13 + ot],
                )
                nc.sync.dma_start(
                    out_d[ot * 128:(ot + 1) * 128, nq:nq + 512],
                    outT_sb[:, ot, nq:nq + 512],
                )

            # ---- wave 5: nq=512 scores first (pv(4) fillers), then nq=0
            # scores (pv(5,512)/pv(4,512) fillers + first e units), so every
            # pv normalize chain hides under score pairs or e units ----
            _, p4 = prev_strips
            strips5 = {0: [], 512: []}
            for nk in range(8):
                strips5[512].append(score_pair(5, 512, nk))
                if nk == 1:
                    pv_group(4, 0, 0, 0, p4[0])
                elif nk == 3:
                    pv_group(4, 0, 1, 64, p4[0])
                elif nk == 5:
                    pv_group(4, 512, 0, 0, p4[512])
            for nk in range(8):
                strips5[0].append(score_pair(5, 0, nk))
                if nk == 0:
                    pv_group(5, 512, 0, 0, strips5[512])
                elif nk == 1:
                    pv_group(5, 512, 1, 64, strips5[512])
                elif nk == 3:
                    pv_group(4, 512, 1, 64, p4[512])
                elif nk == 6:
                    e_unit(0, 512)
                elif nk == 7:
                    e_unit(1, 512)
            e_unit(2, 512)
            pv_group(5, 0, 0, 0, strips5[0])
            e_unit(3, 512)
            pv_group(5, 0, 1, 64, strips5[0])
            e_unit(4, 512)
            e_unit(5, 512)
            for ot in range(CCH):
                e_unit(ot, 0)

    nc.compile()
    return nc


def _get_nc():
    if "nc" not in _CACHE:
        _CACHE["nc"] = _build()
    return _CACHE["nc"]


def _x_h(xT):
    """[768, 1024] -> [p, nq, c, 512] flattened fp16."""
    return np.ascontiguousarray(
        xT.reshape(CCH, 128, 2, 512).transpose(1, 2, 0, 3)
    ).reshape(128, CCH * N).astype(np.float16)


def _wqk_h(w):
    """[768, 1536] -> [p, ot, c, 128] flattened fp16."""
    return np.ascontiguousarray(
        w.reshape(CCH, 128, 12, 128).transpose(1, 2, 0, 3)
    ).reshape(128, CCH * 2 * DIM).astype(np.float16)


def _wblk_h(w):
    """[768, 768] -> blk0 [p, c, 512] ++ blk1 [p, c, 256] fp16."""
    b0 = w[:, 0:512].reshape(CCH, 128, 512).transpose(1, 0, 2).reshape(128, -1)
    b1 = w[:, 512:DIM].reshape(CCH, 128, 256).transpose(1, 0, 2).reshape(128, -1)
    return np.ascontiguousarray(
        np.concatenate([b0, b1], axis=1)).astype(np.float16)


def _wp_h(w):
    """[768, 768] -> [p, ot, c, 128] flattened fp16."""
    return np.ascontiguousarray(
        w.reshape(CCH, 128, CCH, 128).transpose(1, 2, 0, 3)
    ).reshape(128, CCH * DIM).astype(np.float16)


def kernel(x, w_qkv, b_qkv, w_proj, b_proj, **run_kwargs):
    x = np.asarray(x, dtype=np.float32)
    w_qkv = np.asarray(w_qkv, dtype=np.float32)
    b_qkv = np.asarray(b_qkv, dtype=np.float32)
    w_proj = np.asarray(w_proj, dtype=np.float32)
    b_proj = np.asarray(b_proj, dtype=np.float32)

    # Host-side layout prep (no arithmetic beyond folding the 1/sqrt(d) scale
    # into the q projection).
    w_qk = w_qkv[: 2 * DIM].copy()
    b_qk = b_qkv[: 2 * DIM].copy()
    w_qk[:DIM] *= SCALE
    b_qk[:DIM] *= SCALE
    wqk_p = _wqk_h(w_qk.T)                             # [128, 6*1536]
    wv_p = _wblk_h(w_qkv[2 * DIM:].T)
    wp_p = _wp_h(w_proj.T)
    biases = np.concatenate(
        [b_qk.reshape(12, 128).T, b_proj.reshape(CCH, 128).T], axis=1)
    biases = np.ascontiguousarray(biases).astype(np.float32)   # [128, 18]
    b_v = b_qkv[2 * DIM:].reshape(1, DIM).astype(np.float16)

    nc = _get_nc()
    in_maps = []
    for b in range(NB):
        in_maps.append({
            "x_p": _x_h(x[b].T),
            "wqk_p": wqk_p,
            "biases": biases,
            "wv_p": wv_p,
            "b_v": b_v,
            "wp_p": wp_p,
        })
    res = run_bass_kernel_spmd(nc, in_maps, core_ids=list(range(NB)), **run_kwargs)
    out = np.stack(
        [res.results[b]["outT"].T for b in range(NB)], axis=0).astype(np.float32)
    if run_kwargs:
        return out, res
    return out


if __name__ == "__main__":
    rng = np.random.default_rng(0)
    x = rng.standard_normal((NB, N, DIM), dtype=np.float32)
    w_qkv = rng.standard_normal((3 * DIM, DIM), dtype=np.float32) * DIM ** -0.5
    b_qkv = rng.standard_normal((3 * DIM,), dtype=np.float32) * 0.02
    w_proj = rng.standard_normal((DIM, DIM), dtype=np.float32) * DIM ** -0.5
    b_proj = rng.standard_normal((DIM,), dtype=np.float32) * 0.02
    out = kernel(x=x, w_qkv=w_qkv, b_qkv=b_qkv, w_proj=w_proj, b_proj=b_proj)
    print("out", out.shape, out.dtype, float(np.abs(out).mean()))
